# revision 68
# baseline (speedup 1.0000x reference)
"""BertSelfAttention (relative_key_query) Trainium2 kernel, 8-core SPMD. v5

Sharding: core c -> (batch b = c//2, head-group hg = c%2, 8 heads each).

Design notes (v5, ~291us HW vs the 408us v3 baseline):

The TRN2 PE clock p-states (0.65 -> 1.2 -> 2.4 GHz after ~3us of continuous
execution, with HAM load-transient sags) make PE idle gaps and engine-load
spikes expensive: v3's steady slots were DVE/ACT-bound (bias merge chain
~3.2-3.6us/slot vs ~2.7us of PE work), so the PE starved, its ramp kept
resetting, and the whole attention loop ran at ~1.2-1.4 GHz.

v5 folds the entire bias merge into PSUM accumulation:

  psS[r, l] = kq (2 matmuls, start)                  ... scoresT
            + 8x matmul(lhsT=Ush-block, rhs=I)       ... U^T (transpose via
                                                         regular matmul, f32)
            + 2x matmul(lhsT=I, rhs=Vsh-slice)       ... V^T
  probs = ACT.Exp(psS + mask)   directly from PSUM

which deletes the DVE bias16/sc chain entirely.  Band evictions are split
DVE/ACT so both stay under the PE's ~2.8us/slot; the PE is then the
bottleneck, stays continuously busy, and holds the high clock.

Startup: ~90 warm-up matmuls keep the PE executing (and ramping) while the
first 3MB of hs+wq stream in (DMA stream ordered hs/wq first); the 25
prologue bands (U0 + U(1,0) / V0 / V1) are spread one-or-two per projection
unit across the k/qk/v phases so the eviction+shear+DMA load ramps smoothly
into the steady state (abrupt load steps trigger a ~50us HAM clock sag).
U bands run one slot ahead of v3's schedule; q/k proj biases are applied in
the PSUM eviction (Identity activation, per-partition bias) instead of a
9th contraction chunk; ctx matmuls use 128-wide lhsT (padded v) so the
accumulating output has full-partition writes; the output descale is one
batched reciprocal + one broadcast DVE multiply.

k and the q-side distance table are pre-scaled by 1/8 host-side.
"""

import numpy as np
import ml_dtypes
from contextlib import ExitStack

import concourse.bass as bass
import concourse.mybir as mybir
import concourse.tile as tile
from concourse.masks import make_identity

bf16 = ml_dtypes.bfloat16
F32 = mybir.dt.float32
BF16 = mybir.dt.bfloat16

B, S, H = 4, 1024, 1024
NH, HD = 16, 64
MAXPOS = 1024
NCORES = 8
HPC = 8          # heads per core
DHC = HPC * HD   # 512 out-dims per core
KA = H + 8       # augmented contraction (bias fold), 1032
NT = S // 128    # 8 tiles of 128 along sequence
BW = 1152        # band width
VW = 520         # v block width: 8 heads x 65 (64 dims + ones col)
N_WARM = 150     # PE warm-up matmuls covering the initial DMA stream

Exp = mybir.ActivationFunctionType.Exp
Copy = mybir.ActivationFunctionType.Copy
Ident = mybir.ActivationFunctionType.Identity


def _emit(tc, io):
    nc = tc.nc
    ctx = ExitStack()
    with ctx:
        # ---------------- persistent tiles ----------------
        pers = ctx.enter_context(tc.tile_pool(name="pers", bufs=1))
        PT_sb = pers.tile([128, 2048], BF16)
        PrT_sb = pers.tile([128, 2048], BF16)
        ident = pers.tile([128, 128], BF16)
        mask_sb = pers.tile([128, 8], F32)
        qT_sb = pers.tile([128, 4 * 1024], BF16)   # dh-chunk c at cols 1024c
        kT_sb = pers.tile([128, 4 * 1024], BF16)
        # +63 pad cols so head-7 ctx can use 128-wide lhsT slices
        v_sb = pers.tile([128, NT * VW + 63], BF16)  # r-chunk rc at cols VW*rc

        make_identity(nc, ident[:])

        # long-lived pools (created before proj pools so proj can close LIFO)
        bsb = ctx.enter_context(tc.tile_pool(name="bsb", bufs=1))
        work = ctx.enter_context(tc.tile_pool(name="work", bufs=2))
        pBB = ctx.enter_context(tc.tile_pool(name="pBB", bufs=6, space="PSUM"))
        pC = ctx.enter_context(tc.tile_pool(name="pC", bufs=2, space="PSUM"))

        # ---------------- projections ----------------
        outd = io["out"]
        hsd, wqd, wkd, wvd = io["hsT"], io["wqT"], io["wkT"], io["wvT"]

        pj = ctx.enter_context(tc.tile_pool(name="pj", bufs=1))
        hs_m = pj.tile([128, 8 * 1024], BF16)   # k-chunk kc at cols 1024kc
        hs_t = pj.tile([8, 1024], BF16)
        wv_m = pj.tile([128, 8 * VW], BF16)
        wv_t = pj.tile([8, VW], BF16)

        def load_w_half(wm, wd, wcols, half, q=None):
            o = half * 4
            (q or nc.sync).dma_start(
                wm[:, 4 * wcols * half:4 * wcols * half + 4 * wcols],
                bass.AP(wd.ap().tensor, o * 128 * wcols,
                        [[wcols, 128], [128 * wcols, 4], [1, wcols]]))

        def load_hs_half(half):
            o = half * 4
            nc.sync.dma_start(
                hs_m[:, 4096 * half:4096 * half + 4096],
                bass.AP(hsd.ap().tensor, o * 128 * 1024,
                        [[1024, 128], [128 * 1024, 4], [1, 1024]]))

        # ---------------- band machinery ----------------
        ush_st = {}  # h -> Ush
        vsh_st = {}  # h -> Vsh
        rt_st = {}   # h -> dict(ctxH, pending)
        ct_st = {}   # h -> cT

        def emit_band(hn, t, uside, pro=False, defer_evict=False,
                      act_heavy=False):
            if t == 0:
                d, tag, bufs = (ush_st, "Ush", 2) if uside else (vsh_st, "Vsh", 3)
                d[hn] = bsb.tile([128, NT * 1024], BF16, tag=tag, name=tag,
                                 bufs=bufs)
            bb = bsb.tile([128, BW], BF16,
                          tag=("Ub" if uside else "Vb"), name="bb", bufs=2)
            ps0 = pBB.tile([128, 512], F32, tag="B", name="bps0")
            ps1 = pBB.tile([128, 512], F32, tag="B", name="bps1")
            pst = pBB.tile([128, 128], F32, tag="B", name="bpst")

            hc, ho = hn // 2, 64 * (hn % 2)
            s0 = 896 - 128 * t
            src_sb, tbl = (qT_sb, PrT_sb) if uside else (kT_sb, PT_sb)
            lhsT = src_sb[ho:ho + 64,
                          1024 * hc + 128 * t:1024 * hc + 128 * t + 128]
            for ps, w, o in ((ps0, 512, 0), (ps1, 512, 512), (pst, 128, 1024)):
                nc.tensor.matmul(ps[:], lhsT,
                                 tbl[ho:ho + 64, s0 + o:s0 + o + w],
                                 tile_position=(ho, 0))

            def evict():
                if pro:
                    # prologue: ACT is idle -> first halves + small on ACT
                    nc.scalar.copy(bb[:, 0:512], ps0[:])
                    nc.vector.tensor_copy(bb[:, 512:1024], ps1[:])
                    nc.scalar.copy(bb[:, 1024:BW], pst[:])
                elif uside:
                    nc.vector.tensor_scalar_mul(bb[:, 0:512], ps0[:], 1.0)
                    nc.vector.tensor_scalar_mul(bb[:, 512:1024], ps1[:], 1.0)
                    nc.vector.tensor_copy(bb[:, 1024:BW], pst[:])
                elif act_heavy:
                    # tail slot: DVE carries the descale work, so give ACT
                    # the whole V eviction
                    nc.scalar.copy(bb[:, 0:512], ps0[:])
                    nc.scalar.copy(bb[:, 512:1024], ps1[:])
                    nc.vector.tensor_copy(bb[:, 1024:BW], pst[:])
                else:
                    nc.scalar.copy(bb[:, 0:512], ps0[:])
                    nc.vector.tensor_copy(bb[:, 512:1024], ps1[:])
                    nc.vector.tensor_copy(bb[:, 1024:BW], pst[:])
                sap = bb[:]
                diag = bass.AP(sap.tensor, sap.offset + 127,
                               [[BW - 1, 128], [1, 1024]])
                dst_sh = (ush_st[hn] if uside else vsh_st[hn])
                (nc.sync if uside else nc.scalar).dma_start(
                    dst_sh[:, 1024 * t:1024 * t + 1024], diag)

            if defer_evict:
                return evict
            evict()

        # prologue bands, spread through the qk/v projection units so the
        # eviction+shear load ramps up gradually: U(0) first (head-0 slots
        # need all of Ush(0)), then V(0), V(1).
        pro_bands = ([(0, t, True) for t in range(NT)] + [(1, 0, True)] +
                     [(0, t, False) for t in range(NT)] +
                     [(1, t, False) for t in range(NT)])
        pro_iter = iter(pro_bands)

        with tc.tile_pool(name="pjqk", bufs=1) as pjqk:
            wq_m = pjqk.tile([128, 8 * DHC], BF16)
            wk_m = pjqk.tile([128, 8 * DHC], BF16)
            bqk_bf = pjqk.tile([128, 8], BF16)
            bqk = pjqk.tile([128, 8], F32)

            # DMA stream order = consumption order; big stream on sync,
            # small tails on scalar.
            load_hs_half(0)
            load_w_half(wq_m, wqd, DHC, 0)
            load_hs_half(1)
            load_w_half(wq_m, wqd, DHC, 1)
            load_w_half(wk_m, wkd, DHC, 0)
            load_w_half(wk_m, wkd, DHC, 1)
            nc.sync.dma_start(PrT_sb[0:64, :], io["PrT"][:])
            nc.sync.dma_start(PrT_sb[64:128, :], io["PrT"][:])
            nc.sync.dma_start(PT_sb[0:64, :], io["PT"][:])
            nc.sync.dma_start(PT_sb[64:128, :], io["PT"][:])
            load_w_half(wv_m, wvd, VW, 0)
            load_w_half(wv_m, wvd, VW, 1)
            nc.scalar.dma_start(mask_sb[:], io["maskT"][:])
            nc.scalar.dma_start(hs_t[:], hsd.ap()[1024:1032, :])
            nc.scalar.dma_start(wv_t[:], wvd.ap()[1024:1032, 0:VW])
            # q/k bias rows -> per-partition bias columns (col 2c+{0=q,1=k})
            for c in range(4):
                nc.scalar.dma_start(bqk_bf[:, 2 * c:2 * c + 1],
                                    wqd.ap()[1024:1025, 128 * c:128 * c + 128])
                nc.scalar.dma_start(bqk_bf[:, 2 * c + 1:2 * c + 2],
                                    wkd.ap()[1024:1025, 128 * c:128 * c + 128])
            nc.vector.tensor_copy(bqk[:], bqk_bf[:])

            # ---------------- PE warm-up ----------------
            # Independent back-to-back matmuls keep the PE "continuously
            # executing" from t~0 so the 2.4GHz p-state is reached while
            # the projection inputs stream in.  The wide phase reads the
            # first hs chunk so it gates on (and then tracks) the DMA.
            for _ in range(50):
                ps = pBB.tile([128, 512], F32, tag="B", name="warm")
                nc.tensor.matmul(ps[:, 0:128], ident[:], ident[:])
            for _ in range(38):
                ps = pBB.tile([128, 512], F32, tag="B", name="warm")
                nc.tensor.matmul(ps[:], ident[:], hs_m[:, 0:512])

            def qk_unit(wm, bcol, dst, c, th):
                ps = pBB.tile([128, 512], F32, tag="B", name="pps")
                for kc in range(8):
                    nc.tensor.matmul(
                        ps[:],
                        wm[:, 512 * kc + 128 * c:512 * kc + 128 * c + 128],
                        hs_m[:, 1024 * kc + 512 * th:1024 * kc + 512 * th + 512],
                        start=(kc == 0), stop=(kc == 7))
                # bias folded into the eviction (bias rows never hit the PE)
                nc.scalar.activation(
                    dst[:, 1024 * c + 512 * th:1024 * c + 512 * th + 512],
                    ps[:], Ident, bias=bqk[:, bcol:bcol + 1])

            for c in (0, 1):
                for th in range(2):
                    qk_unit(wq_m, 2 * c, qT_sb, c, th)
            for c in (0, 1):
                for th in range(2):
                    qk_unit(wk_m, 2 * c + 1, kT_sb, c, th)
                    emit_band(*next(pro_iter), pro=True)
            for wm, boff, dst in ((wq_m, 0, qT_sb), (wk_m, 1, kT_sb)):
                for c in (2, 3):
                    for th in range(2):
                        qk_unit(wm, 2 * c + boff, dst, c, th)
                        emit_band(*next(pro_iter), pro=True)

        # ---------------- v projection + remaining prologue bands -------
        for rc in range(8):
            psa = pBB.tile([128, 512], F32, tag="B", name="pps")
            psb = pBB.tile([128, 8], F32, tag="B", name="ppsb")
            for kc in range(8):
                lhsT = hs_m[:, 1024 * kc + 128 * rc:1024 * kc + 128 * rc + 128]
                nc.tensor.matmul(psa[:], lhsT,
                                 wv_m[:, VW * kc:VW * kc + 512],
                                 start=(kc == 0), stop=False)
                nc.tensor.matmul(psb[:], lhsT,
                                 wv_m[:, VW * kc + 512:VW * kc + VW],
                                 start=(kc == 0), stop=False)
            nc.tensor.matmul(psa[:], hs_t[:, 128 * rc:128 * rc + 128],
                             wv_t[:, 0:512], start=False, stop=True)
            nc.tensor.matmul(psb[:], hs_t[:, 128 * rc:128 * rc + 128],
                             wv_t[:, 512:VW], start=False, stop=True)
            nc.vector.tensor_copy(v_sb[:, VW * rc:VW * rc + 512], psa[:])
            nc.vector.tensor_copy(v_sb[:, VW * rc + 512:VW * rc + VW], psb[:])
            for _, b in zip(range(2), pro_iter):
                emit_band(*b, pro=True)
        for b in pro_iter:
            emit_band(*b, pro=True)

        # ---------------- per-head attention ----------------
        def emit_ctx(h, rt, probs, ctxH):
            # 128-wide lhsT (spills into the next head's v columns; the
            # extra out rows 65..127 are garbage and never read) so the
            # matmul runs with full-partition output
            for j in range(2):
                nc.tensor.matmul(
                    ctxH[j][:, :],
                    v_sb[:, VW * rt + 65 * h:VW * rt + 65 * h + 128],
                    probs[:, 512 * j:512 * j + 512],
                    start=(rt == 0), stop=(rt == NT - 1),
                    skip_group_check=True)

        def emit_slot_scores(h, rt):
            hc, ho = h // 2, 64 * (h % 2)
            Ush = ush_st[h]
            Vsh = vsh_st[h]
            if rt == 0:
                rt_st[h] = {
                    "ctxH": [pC.tile([128, 512], F32, tag="C", name=f"ctx{j}")
                             for j in range(2)],
                    "pending": []}
            sth = rt_st[h]
            psS0 = pBB.tile([128, 512], F32, tag="B", name="psS0")
            psS1 = pBB.tile([128, 512], F32, tag="B", name="psS1")
            lhsT = kT_sb[ho:ho + 64,
                         1024 * hc + 128 * rt:1024 * hc + 128 * rt + 128]
            for half, psS in ((0, psS0), (1, psS1)):
                nc.tensor.matmul(
                    psS[:], lhsT,
                    qT_sb[ho:ho + 64,
                          1024 * hc + 512 * half:1024 * hc + 512 * half + 512],
                    start=True, stop=False, skip_group_check=True)
                for lt in range(4 * half, 4 * half + 4):
                    nc.tensor.matmul(
                        psS[:, 128 * (lt % 4):128 * (lt % 4) + 128],
                        Ush[:, 1024 * lt + 128 * rt:1024 * lt + 128 * rt + 128],
                        ident[:], start=False, stop=False,
                        skip_group_check=True)
                nc.tensor.matmul(
                    psS[:], ident[:],
                    Vsh[:, 1024 * rt + 512 * half:1024 * rt + 512 * half + 512],
                    start=False, stop=True, skip_group_check=True)
            sth["cur"] = (psS0, psS1)

        def emit_slot_exp(h, rt):
            sth = rt_st[h]
            psS0, psS1 = sth.pop("cur")
            probs = work.tile([128, 1024], BF16, tag="probs", bufs=3)
            nc.scalar.activation(probs[:, 0:512], psS0[:], Exp,
                                 bias=mask_sb[:, rt:rt + 1])
            nc.scalar.activation(probs[:, 512:1024], psS1[:], Exp,
                                 bias=mask_sb[:, rt:rt + 1])
            sth["pending"].append((rt, probs))
            if rt >= 2:
                prt, pp = sth["pending"][rt - 2]
                emit_ctx(h, prt, pp, sth["ctxH"])

        def emit_late_ctx(h, rt):
            sth = rt_st[h]
            prt, pp = sth["pending"][rt]
            emit_ctx(h, prt, pp, sth["ctxH"])

        def emit_head_close(h):
            sth = rt_st.pop(h)
            cT = work.tile([128, 1024], BF16, tag="cT", bufs=2)
            # split across ACT/DVE: slot i=1's ACT queue already carries
            # both Exps plus the deferred V eviction
            nc.scalar.copy(cT[0:65, 0:512], sth["ctxH"][0][0:65, :])
            nc.vector.tensor_copy(cT[0:65, 512:1024], sth["ctxH"][1][0:65, :])
            ct_st[h] = cT

        def emit_tail(h):
            cT = ct_st.pop(h)
            # 66-col block stride keeps each PSUM write 4-byte aligned
            outT = pBB.tile([128, 528], BF16, tag="B", name="outT")
            for lt in range(NT):
                nc.tensor.matmul(outT[:, 66 * lt:66 * lt + 65],
                                 cT[0:65, 128 * lt:128 * lt + 128],
                                 ident[0:65, 0:65], is_transpose=True)
            rcp = work.tile([128, 8], F32, tag="rcp")
            outsb = work.tile([128, 512], F32, tag="outsb")
            dap = outT[:]
            den = bass.AP(dap.tensor, dap.offset + 64, [[528, 128], [66, 8]])
            nc.vector.reciprocal(rcp[:], den)
            rap = rcp[:]
            oap = outsb[:]
            nc.vector.tensor_tensor(
                bass.AP(oap.tensor, oap.offset, [[512, 128], [64, 8], [1, 64]]),
                bass.AP(dap.tensor, dap.offset, [[528, 128], [66, 8], [1, 64]]),
                bass.AP(rap.tensor, rap.offset, [[8, 128], [1, 8], [0, 64]]),
                mybir.AluOpType.mult)
            oap = outsb[:]
            src = bass.AP(oap.tensor, oap.offset, [[512, 128], [64, 8], [1, 64]])
            dst = bass.AP(outd.ap().tensor, 64 * h,
                          [[512, 128], [128 * 512, 8], [1, 64]])
            nc.sync.dma_start(dst, src)

        for h in range(HPC):
            for i in range(NT):
                # U bands run one slot ahead (U(h+1,i+1) at slot (h,i)) so
                # the last shear lands a full slot before head h+1 reads it
                if h + 1 < HPC and i < NT - 1:
                    emit_band(h + 1, i + 1, True)
                elif h + 2 < HPC and i == NT - 1:
                    emit_band(h + 2, 0, True)
                # defer the V eviction past the Exps (ACT runs Exp0, Exp1,
                # Vcopy) except in the tail slot i=2 where ACT is loaded
                vthunk = None
                if h + 2 < HPC:
                    vthunk = emit_band(h + 2, i, False, defer_evict=(i != 2),
                                       act_heavy=(i == 2))
                emit_slot_scores(h, i)
                emit_slot_exp(h, i)
                if vthunk is not None:
                    vthunk()
                if h > 0:
                    if i == 0:
                        emit_late_ctx(h - 1, NT - 2)
                    elif i == 1:
                        emit_late_ctx(h - 1, NT - 1)
                        emit_head_close(h - 1)
                    elif i == 2:
                        emit_tail(h - 1)
        emit_late_ctx(HPC - 1, NT - 2)
        emit_late_ctx(HPC - 1, NT - 1)
        emit_head_close(HPC - 1)
        emit_tail(HPC - 1)


def build_module():
    from concourse import bacc
    nc = bacc.Bacc("TRN2", target_bir_lowering=False)
    io = {
        "hsT": nc.dram_tensor("hsT", [KA, S], BF16, kind="ExternalInput"),
        "wqT": nc.dram_tensor("wqT", [KA, DHC], BF16, kind="ExternalInput"),
        "wkT": nc.dram_tensor("wkT", [KA, DHC], BF16, kind="ExternalInput"),
        "wvT": nc.dram_tensor("wvT", [KA, VW], BF16, kind="ExternalInput"),
        "PT": nc.dram_tensor("PT", [64, 2048], BF16, kind="ExternalInput"),
        "PrT": nc.dram_tensor("PrT", [64, 2048], BF16, kind="ExternalInput"),
        "maskT": nc.dram_tensor("maskT", [128, 8], F32, kind="ExternalInput"),
        "out": nc.dram_tensor("out", [S, DHC], F32, kind="ExternalOutput"),
    }
    with tile.TileContext(nc) as tc:
        _emit(tc, io)
    nc.compile()
    return nc


def shard_inputs(hidden_states, attention_mask, wq, bq, wk, bk, wv, bv, dist_emb):
    """Full fp32 inputs -> per-core in_maps (bf16 where appropriate).

    k and the q-side distance table are pre-scaled by 1/8 so scores, q-bias
    and k-bias all land in PSUM already divided by sqrt(HD)."""
    hidden_states = np.asarray(hidden_states, np.float32)
    attention_mask = np.asarray(attention_mask, np.float32)
    wq, bq = np.asarray(wq, np.float32), np.asarray(bq, np.float32)
    wk = np.asarray(wk, np.float32) * 0.125
    bk = np.asarray(bk, np.float32) * 0.125
    wv, bv = np.asarray(wv, np.float32), np.asarray(bv, np.float32)
    dist_emb = np.asarray(dist_emb, np.float32)

    PT = np.zeros((64, 2048), bf16)
    PT[:, :2047] = dist_emb.T.astype(bf16)
    PrT = np.zeros((64, 2048), bf16)
    PrT[:, :2047] = (dist_emb[::-1].T * 0.125).astype(bf16)

    in_maps = []
    for c in range(NCORES):
        b, hg = c // 2, c % 2
        sl = slice(DHC * hg, DHC * (hg + 1))

        hsT = np.zeros((KA, S), bf16)
        hsT[:H] = hidden_states[b].T.astype(bf16)
        hsT[H] = bf16(1.0)

        wqT = np.zeros((KA, DHC), bf16)
        wqT[:H] = wq[sl].T.astype(bf16)
        wqT[H] = bq[sl].astype(bf16)
        wkT = np.zeros((KA, DHC), bf16)
        wkT[:H] = wk[sl].T.astype(bf16)
        wkT[H] = bk[sl].astype(bf16)

        wvT = np.zeros((KA, VW), bf16)
        for h in range(HPC):
            cs = 65 * h
            wvT[:H, cs:cs + 64] = wv[DHC * hg + 64 * h:DHC * hg + 64 * h + 64].T.astype(bf16)
            wvT[H, cs:cs + 64] = bv[DHC * hg + 64 * h:DHC * hg + 64 * h + 64].astype(bf16)
            wvT[H, cs + 64] = bf16(1.0)

        maskT = np.ascontiguousarray(
            attention_mask[b, 0, 0].reshape(8, 128).T).astype(np.float32)

        in_maps.append({"hsT": hsT, "wqT": wqT, "wkT": wkT, "wvT": wvT,
                        "PT": PT.copy(), "PrT": PrT.copy(), "maskT": maskT})
    return in_maps


def assemble_output(results):
    out = np.zeros((B, S, H), np.float32)
    for c in range(NCORES):
        b, hg = c // 2, c % 2
        out[b, :, DHC * hg:DHC * (hg + 1)] = results[c]["out"]
    return out


_NC_CACHE = {}


def kernel(**inputs):
    from concourse import bass_utils
    if "nc" not in _NC_CACHE:
        _NC_CACHE["nc"] = build_module()
    nc = _NC_CACHE["nc"]
    in_maps = shard_inputs(**inputs)
    res = bass_utils.run_bass_kernel_spmd(nc, in_maps, core_ids=list(range(NCORES)))
    return assemble_output(res.results)
